# revision 7
# baseline (speedup 1.0000x reference)
"""Trainium2 Bass kernel for nn_FLASH_ShareA_FFConvM.

Strategy: data-parallel over (batch, seq-half): 8 cores, each handling 4096
tokens (16 local-attention chunks of 256). All weights replicated. Per core:

  phase A (token-major): LayerNorm stats + normalized x, bf16 copy,
    DMA-transpose into channel-major xs^T [512c x 4224t] (col j = token j-128;
    128-row halo tile at the front provides the token-shift source).
    The "shift first half of channels by one token" becomes a column-offset
    AP on xs^T c-chunks 0..1.
  phase B (per chunk g): qk^T = wqk^T @ xs_sh^T; per-chunk mean stats ->
    OffsetScale -> q/k/qs/ks (ACT per-partition scale+bias, shift = col AP);
    sim^T[j,i] per head via PE (K=64); mask+exp; softmax denominator via
    ones-matmul; reciprocal broadcast via DRAM bounce; attn = exp * rec;
    V-matmul out^T[e,(h,i)] with token-major hidden as stationary;
    silu*gate -> og^T; fin[t,d] PSUM accumulates xs_sh@W_comb (folded
    hidden@w_out[:512]) + og@w_out[512:]; finalize y = fin*silu(vgate) + xs_sh
    with fp32 xs recomputed from saved LN stats (token-shift via SBUF DMA).
"""

import sys

sys.path.insert(0, "/opt/trn_rl_repo")

import numpy as np
import ml_dtypes
from contextlib import ExitStack

import concourse.bass as bass
import concourse.tile as tile
from concourse import bacc, mybir

F32 = mybir.dt.float32
BF16 = mybir.dt.bfloat16
AX = mybir.AxisListType
ALU = mybir.AluOpType
ACTF = mybir.ActivationFunctionType

B, SEQ, DIM = 4, 8192, 512
G, QK = 32, 128
CHUNK = SEQ // G          # 256 tokens per attention chunk
HD = QK // 4              # 32 (softmax scale dim, per source)
SCALE = float(HD) ** -0.5
HID = DIM                 # 512
EPS = 1e-5
N_CORES = 8
T_CORE = SEQ // 2         # 4096 tokens per core
NEG = -1.0e30

BF = ml_dtypes.bfloat16


def build_core_program(ctx: ExitStack, tc, aps, n_tok, apply_g, apply_b,
                       pe_transpose=False, silu_native=True):
    """Emit the per-core program. aps: dict name -> bass.AP (DRAM)."""
    nc = tc.nc
    n_tiles = n_tok // 128            # 128-token tiles (excl. halo tile)
    n_chunks = n_tok // CHUNK
    n_pad = n_tok + 128               # halo tile rows 0..127 (token j-128)

    xp = aps["xp"]; yout = aps["y"]

    # ---------------- pools ----------------
    consts = ctx.enter_context(tc.tile_pool(name="consts", bufs=1))
    persist = ctx.enter_context(tc.tile_pool(name="persist", bufs=1))
    work = ctx.enter_context(tc.tile_pool(name="work", bufs=1))
    psum = ctx.enter_context(tc.tile_pool(name="psum", bufs=1, space="PSUM"))
    dram = ctx.enter_context(tc.tile_pool(name="dram", bufs=1, space="DRAM"))

    # ---------------- constants into SBUF ----------------
    def cload(name, shape, dtype):
        """Load a [R, C] DRAM const into SBUF; R>128 folds to [128, R//128, C]
        (row r = cc*128 + p -> tile[p, cc, :]), returning slices via [:, cc, :].
        """
        if shape[0] > 128:
            k = shape[0] // 128
            t = consts.tile([128, k, shape[1]], dtype, name=f"c_{name}",
                            tag=f"c_{name}")
            nc.sync.dma_start(t[:], aps[name].rearrange("(k p) c -> p k c",
                                                        p=128))
        else:
            t = consts.tile(shape, dtype, name=f"c_{name}", tag=f"c_{name}")
            nc.sync.dma_start(t[:], aps[name])
        return t

    wqk = cload("wqk", [512, 128], BF16)      # [128c, 4cc, 128d]
    whvg = cload("whvg", [512, 1024], BF16)   # moving: cols 0:512 hid, 512: vgate
    wga = cload("wga", [512, 512], BF16)      # attn gate; lhsT tiles
    wcomb = cload("wcomb", [512, 512], BF16)  # moving rhs for fin part1
    woa = cload("woa", [2048, 512], BF16)     # moving rhs for fin part2
    g4t = cload("g4t", [128, 4], F32)         # pre-divided by CHUNK
    b4t = cload("b4t", [128, 4], F32)
    g2t = cload("g2t", [128, 2], F32)         # pre-divided by CHUNK
    b2t = cload("b2t", [128, 2], F32)
    maskt = cload("maskt", [256, 256], F32)   # [j, i]: 0 if j<=i else NEG
    if apply_g:
        lng = cload("lng", [128, 512], F32)
    if apply_b:
        lnb = cload("lnb", [128, 512], F32)
    identb = cload("identb", [128, 128], BF16)

    ones_bf = consts.tile([128, 1], BF16, name="ones_bf", tag="ones_bf")
    nc.vector.memset(ones_bf[:], 1.0)
    epsb = consts.tile([128, 1], F32, name="epsb", tag="epsb")
    nc.vector.memset(epsb[:], EPS)

    # ---------------- persistent state ----------------
    # channel-major normalized+g-scaled x, bf16; col j = token (j - 128)
    xsT = [persist.tile([128, n_pad], BF16, name=f"xsT{cc}", tag=f"xsT{cc}")
           for cc in range(4)]
    # per-LN-tile stats: cols 2i (mean), 2i+1 (rstd); tile i covers xp rows
    # [i*128, (i+1)*128)
    stats = persist.tile([128, 2 * (n_tiles + 1)], F32, name="stats",
                         tag="stats")

    # ---------------- phase A: LN + transpose ----------------
    for i in range(n_tiles + 1):
        x_t = work.tile([128, 512], F32, name=f"xa{i}", tag="xa", bufs=3)
        nc.sync.dma_start(x_t[:], xp[i * 128:(i + 1) * 128, :])

        bns = work.tile([128, 6], F32, name=f"bns{i}", tag="bns", bufs=2)
        nc.vector.bn_stats(out=bns[:], in_=x_t[:])
        mv = work.tile([128, 2], F32, name=f"mv{i}", tag="mv", bufs=2)
        nc.vector.bn_aggr(out=mv[:], in_=bns[:])
        nc.vector.tensor_copy(out=stats[:, 2 * i:2 * i + 1], in_=mv[:, 0:1])
        # rstd = 1/sqrt(var+eps)
        sd = work.tile([128, 1], F32, name=f"sd{i}", tag="sd", bufs=2)
        nc.scalar.activation(out=sd[:], in_=mv[:, 1:2], func=ACTF.Sqrt,
                             bias=epsb[:])
        nc.vector.reciprocal(out=sd[:], in_=sd[:])
        nc.vector.tensor_copy(out=stats[:, 2 * i + 1:2 * i + 2], in_=sd[:])

        xs_bf = work.tile([128, 512], BF16, name=f"xsbf{i}", tag="xsbf",
                          bufs=3)
        if apply_g or apply_b:
            xs_f = work.tile([128, 512], F32, name=f"xsf{i}", tag="xsf",
                             bufs=2)
            nc.vector.tensor_scalar(out=xs_f[:], in0=x_t[:],
                                    scalar1=mv[:, 0:1], scalar2=sd[:],
                                    op0=ALU.subtract, op1=ALU.mult)
            if apply_g:
                nc.vector.tensor_mul(out=(xs_f if apply_b else xs_bf)[:],
                                     in0=xs_f[:], in1=lng[:])
            if apply_b:
                nc.vector.tensor_add(out=xs_bf[:], in0=xs_f[:], in1=lnb[:])
        else:
            nc.vector.tensor_scalar(out=xs_bf[:], in0=x_t[:],
                                    scalar1=mv[:, 0:1], scalar2=sd[:],
                                    op0=ALU.subtract, op1=ALU.mult)

        # transpose 4x [128,128] into xsT[cc][:, i*128:(i+1)*128]
        for cc in range(4):
            dst = xsT[cc][:, i * 128:(i + 1) * 128]
            src = xs_bf[:, cc * 128:(cc + 1) * 128]
            if pe_transpose:
                tp = psum.tile([128, 128], BF16, name=f"tp{i}_{cc}",
                               tag="mm_s", bufs=2)
                nc.tensor.transpose(tp[:], src, identb[:])
                nc.vector.tensor_copy(out=dst, in_=tp[:])
            else:
                nc.sync.dma_start(dst, src, transpose=True)

    # ---------------- halo xs (token-major, fp32) for residual shift -------
    def emit_xs_tok(ti):
        """Recompute fp32 token-major xs for xp tile ti (rows ti*128..)."""
        x2 = work.tile([128, 512], F32, name=f"xr{ti}", tag="xr", bufs=3)
        nc.sync.dma_start(x2[:], xp[ti * 128:(ti + 1) * 128, :])
        xs = work.tile([128, 512], F32, name=f"xstok{ti}", tag="xstok",
                       bufs=3)
        nc.vector.tensor_scalar(out=xs[:], in0=x2[:],
                                scalar1=stats[:, 2 * ti:2 * ti + 1],
                                scalar2=stats[:, 2 * ti + 1:2 * ti + 2],
                                op0=ALU.subtract, op1=ALU.mult)
        if apply_g:
            nc.vector.tensor_mul(out=xs[:], in0=xs[:], in1=lng[:])
        if apply_b:
            nc.vector.tensor_add(out=xs[:], in0=xs[:], in1=lnb[:])
        return xs

    prev_xs = emit_xs_tok(0)   # halo tile; only row 127 is ever read

    # ---------------- phase B: chunks ----------------
    for g in range(n_chunks):
        colU = 128 + g * CHUNK        # unshifted start col in xsT
        colS = colU - 1               # shifted (first-half channels)

        def xsh(cc, col0, width):
            c0 = col0 - 1 if cc < 2 else col0
            return xsT[cc][:, c0:c0 + width]

        # --- B1: qk^T [128qk, 256] ---
        qkps = psum.tile([128, 256], F32, name=f"qkps{g}", tag="mm_s", bufs=2)
        for cc in range(4):
            nc.tensor.matmul(qkps[:], wqk[:, cc, :],
                             xsh(cc, colU, 256),
                             start=(cc == 0), stop=(cc == 3))
        qkT = work.tile([128, 256], F32, name=f"qkT{g}", tag="qkT", bufs=2)
        nc.vector.tensor_copy(out=qkT[:], in_=qkps[:])

        # --- B2: per-chunk stats -> offsets/scales [128,1] each ---
        qsum = work.tile([128, 1], F32, name=f"qsum{g}", tag="qsum", bufs=2)
        nc.vector.tensor_reduce(out=qsum[:], in_=qkT[:], axis=AX.X,
                                op=ALU.add)
        offs = work.tile([128, 6], F32, name=f"offs{g}", tag="offs", bufs=2)
        for idx in range(4):   # 0 qoff, 1 koff, 2 qsc, 3 ksc
            nc.vector.tensor_scalar(out=offs[:, idx:idx + 1], in0=qsum[:],
                                    scalar1=g4t[:, idx:idx + 1],
                                    scalar2=b4t[:, idx:idx + 1],
                                    op0=ALU.mult, op1=ALU.add)
        for idx in range(2):   # 4 qsoff, 5 ksoff
            nc.vector.tensor_scalar(out=offs[:, 4 + idx:5 + idx],
                                    in0=qsum[:],
                                    scalar1=g2t[:, idx:idx + 1],
                                    scalar2=b2t[:, idx:idx + 1],
                                    op0=ALU.mult, op1=ALU.add)
        qoff, koff, qsc, ksc, qsoff, ksoff = (offs[:, i:i + 1]
                                              for i in range(6))

        # --- B3: q/k/qs/ks channel-major bf16 [128qk, 256] ---
        qT = work.tile([128, 256], BF16, name=f"qT{g}", tag="qT", bufs=2)
        kT = work.tile([128, 256], BF16, name=f"kT{g}", tag="kT", bufs=2)
        nc.scalar.activation(out=qT[:], in_=qkT[:], func=ACTF.Identity,
                             bias=qoff, scale=qsc)
        nc.scalar.activation(out=kT[:], in_=qkT[:], func=ACTF.Identity,
                             bias=koff, scale=ksc)
        qsT = work.tile([128, 256], BF16, name=f"qsT{g}", tag="qsT", bufs=2)
        ksT = work.tile([128, 256], BF16, name=f"ksT{g}", tag="ksT", bufs=2)
        nc.vector.tensor_copy(out=qsT[:, 0:1], in_=qsoff)
        nc.vector.tensor_copy(out=ksT[:, 0:1], in_=ksoff)
        nc.scalar.activation(out=qsT[:, 1:256], in_=qT[:, 0:255],
                             func=ACTF.Identity, bias=qsoff, scale=qsc)
        nc.scalar.activation(out=ksT[:, 1:256], in_=kT[:, 0:255],
                             func=ACTF.Identity, bias=ksoff, scale=ksc)

        # --- B4: sim^T + mask + exp -> exp[jt] [128j, 4h*256i] bf16 ---
        expt = [work.tile([128, 1024], BF16, name=f"exp{g}_{jt}",
                          tag=f"exp{jt}", bufs=2) for jt in range(2)]
        for h in range(4):
            Q = qT if h < 2 else qsT
            K = kT if h < 2 else ksT
            dr = (h % 2) * 64
            for jt in range(2):
                sim = psum.tile([128, 256], F32, name=f"sim{g}_{h}_{jt}",
                                tag="mm_s", bufs=2)
                nc.tensor.matmul(sim[:],
                                 K[dr:dr + 64, jt * 128:(jt + 1) * 128],
                                 Q[dr:dr + 64, :], start=True, stop=True)
                tmp = work.tile([128, 256], F32, name=f"ms{g}_{h}_{jt}",
                                tag="ms", bufs=3)
                nc.vector.scalar_tensor_tensor(
                    out=tmp[:], in0=sim[:], scalar=SCALE,
                    in1=maskt[:, jt, :],
                    op0=ALU.mult, op1=ALU.add)
                nc.scalar.activation(out=expt[jt][:, h * 256:(h + 1) * 256],
                                     in_=tmp[:], func=ACTF.Exp)

        # --- B5: denominator + reciprocal + broadcast + attn ---
        sums = psum.tile([1, 1024], F32, name=f"sums{g}", tag="hvv", bufs=2)
        for jt in range(2):
            for s in range(2):
                nc.tensor.matmul(sums[0:1, s * 512:(s + 1) * 512], ones_bf[:],
                                 expt[jt][:, s * 512:(s + 1) * 512],
                                 start=(jt == 0), stop=(jt == 1))
        rec = work.tile([1, 1024], F32, name=f"rec{g}", tag="rec", bufs=2)
        nc.vector.reciprocal(out=rec[:], in_=sums[0:1, :])
        recd = dram.tile([1, 1024], F32, name=f"recd{g}", tag="recd", bufs=2)
        nc.sync.dma_start(recd[:], rec[:])
        recb = work.tile([128, 1024], F32, name=f"recb{g}", tag="recb",
                         bufs=2)
        bcast = bass.AP(tensor=recd.tensor, offset=recd.offset,
                        ap=[[0, 128], list(recd.ap)[-1]])
        nc.sync.dma_start(recb[:], bcast)
        attn = [work.tile([128, 1024], BF16, name=f"attn{g}_{jt}",
                          tag=f"attn{jt}", bufs=2) for jt in range(2)]
        for jt in range(2):
            nc.vector.tensor_mul(out=attn[jt][:], in0=expt[jt][:],
                                 in1=recb[:])

        # --- B6: hidden + vgate (token-major) for the chunk's 2 t-tiles ---
        hid_bf = []
        svg_bf = []
        for tt in range(2):
            ti = g * 2 + tt
            colT = 128 + ti * 128
            hv = psum.tile([128, 1024], F32, name=f"hv{g}_{tt}", tag="hvv",
                           bufs=2)
            for cc in range(4):
                for s in range(2):
                    nc.tensor.matmul(
                        hv[:, s * 512:(s + 1) * 512],
                        xsh(cc, colT, 128),
                        whvg[:, cc, s * 512:(s + 1) * 512],
                        start=(cc == 0), stop=(cc == 3))
            hb = work.tile([128, 512], BF16, name=f"hid{g}_{tt}", tag="hid",
                           bufs=4)
            nc.vector.tensor_copy(out=hb[:], in_=hv[:, 0:512])
            sv = work.tile([128, 512], BF16, name=f"svg{g}_{tt}", tag="svg",
                           bufs=4)
            if silu_native:
                nc.scalar.activation(out=sv[:], in_=hv[:, 512:1024],
                                     func=ACTF.Silu)
            else:
                sgt = work.tile([128, 512], F32, name=f"sg{g}_{tt}",
                                tag="sgt", bufs=2)
                nc.scalar.activation(out=sgt[:], in_=hv[:, 512:1024],
                                     func=ACTF.Sigmoid)
                nc.vector.tensor_mul(out=sv[:], in0=sgt[:],
                                     in1=hv[:, 512:1024])
            hid_bf.append(hb)
            svg_bf.append(sv)

        # --- B7: attn gate^T (channel-major) ---
        gate_bf = []
        for ee in range(4):
            gps = psum.tile([128, 256], F32, name=f"g{g}_{ee}", tag="mm_s",
                            bufs=2)
            for cc in range(4):
                nc.tensor.matmul(gps[:],
                                 wga[:, cc, ee * 128:(ee + 1) * 128],
                                 xsh(cc, colU, 256),
                                 start=(cc == 0), stop=(cc == 3))
            gb = work.tile([128, 256], BF16, name=f"gate{g}_{ee}",
                           tag=f"gate{ee}", bufs=2)
            nc.vector.tensor_copy(out=gb[:], in_=gps[:])
            gate_bf.append(gb)

        # --- B8: V matmul + silu + gate -> og^T [128e, 4h*256i] bf16 ---
        og_bf = []
        for ee in range(4):
            vps = psum.tile([128, 1024], F32, name=f"v{g}_{ee}", tag="hvv",
                            bufs=2)
            for jt in range(2):
                for s in range(2):
                    nc.tensor.matmul(vps[:, s * 512:(s + 1) * 512],
                                     hid_bf[jt][:, ee * 128:(ee + 1) * 128],
                                     attn[jt][:, s * 512:(s + 1) * 512],
                                     start=(jt == 0), stop=(jt == 1))
            osl = work.tile([128, 1024], BF16, name=f"osl{g}_{ee}",
                            tag="osl", bufs=2)
            if silu_native:
                nc.scalar.activation(out=osl[:], in_=vps[:], func=ACTF.Silu)
            else:
                sgo = work.tile([128, 1024], F32, name=f"sgo{g}_{ee}",
                                tag="sgo", bufs=2)
                nc.scalar.activation(out=sgo[:], in_=vps[:],
                                     func=ACTF.Sigmoid)
                nc.vector.tensor_mul(out=osl[:], in0=sgo[:], in1=vps[:])
            ob = work.tile([128, 1024], BF16, name=f"og{g}_{ee}",
                           tag=f"og{ee}", bufs=2)
            for h in range(4):
                nc.vector.tensor_mul(out=ob[:, h * 256:(h + 1) * 256],
                                     in0=osl[:, h * 256:(h + 1) * 256],
                                     in1=gate_bf[ee][:])
            og_bf.append(ob)

        # --- B9 + B10: fin PSUM accumulation, then finalize each t-tile ---
        for tt in range(2):
            ti = g * 2 + tt
            colT = 128 + ti * 128
            fin = psum.tile([128, 512], F32, name=f"fin{g}_{tt}", tag="fin",
                            bufs=2)
            for cc in range(4):
                nc.tensor.matmul(fin[:], xsh(cc, colT, 128),
                                 wcomb[:, cc, :],
                                 start=(cc == 0), stop=False)
            for h in range(4):
                for ee in range(4):
                    ff = h * 4 + ee
                    nc.tensor.matmul(
                        fin[:],
                        og_bf[ee][:, h * 256 + tt * 128:h * 256 + tt * 128
                                  + 128],
                        woa[:, ff, :],
                        start=False, stop=(ff == 15))

            xs_cur = emit_xs_tok(ti + 1)
            xsprev = work.tile([128, 256], F32, name=f"xsp{ti}", tag="xsp",
                               bufs=2)
            nc.sync.dma_start(xsprev[1:128, :], xs_cur[0:127, 0:256])
            nc.sync.dma_start(xsprev[0:1, :], prev_xs[127:128, 0:256])
            prev_xs = xs_cur

            y = work.tile([128, 512], F32, name=f"y{ti}", tag="y", bufs=3)
            nc.vector.tensor_mul(out=y[:], in0=fin[:], in1=svg_bf[tt][:])
            nc.vector.tensor_add(out=y[:, 256:512], in0=y[:, 256:512],
                                 in1=xs_cur[:, 256:512])
            nc.vector.tensor_add(out=y[:, 0:256], in0=y[:, 0:256],
                                 in1=xsprev[:])
            nc.sync.dma_start(yout[ti * 128:(ti + 1) * 128, :], y[:])


def make_host_inputs(x, ln_g, ln_b, w_qk, g4, b4, g2, b2, w_hidden, w_gate,
                     w_out, n_tok=T_CORE):
    """Build shared weight arrays + per-core xp slices."""
    x = np.asarray(x, np.float32)
    ln_g = np.asarray(ln_g, np.float32)
    ln_b = np.asarray(ln_b, np.float32)
    apply_g = not np.all(ln_g == 1.0)
    apply_b = bool(np.any(ln_b != 0.0))

    w_hidden = np.asarray(w_hidden, np.float32)
    w_out = np.asarray(w_out, np.float32)
    w_gate = np.asarray(w_gate, np.float32)
    w_qk = np.asarray(w_qk, np.float32)

    wcomb = (w_hidden[:, :HID] @ w_out[:HID, :]).astype(np.float32)

    jj, ii = np.meshgrid(np.arange(256), np.arange(256), indexing="ij")
    maskt = np.where(jj > ii, np.float32(NEG), np.float32(0.0))

    shared = {
        "wqk": w_qk.astype(BF),
        "whvg": np.concatenate([w_hidden[:, :HID], w_gate], axis=1).astype(BF),
        "wga": w_hidden[:, HID:].astype(BF),
        "wcomb": wcomb.astype(BF),
        "woa": w_out[HID:, :].astype(BF),
        "g4t": np.broadcast_to((np.asarray(g4, np.float32) / CHUNK).T,
                               (QK, 4)).copy(),
        "b4t": np.broadcast_to(np.asarray(b4, np.float32).T, (QK, 4)).copy(),
        "g2t": np.broadcast_to((np.asarray(g2, np.float32) / CHUNK).T,
                               (QK, 2)).copy(),
        "b2t": np.broadcast_to(np.asarray(b2, np.float32).T, (QK, 2)).copy(),
        "maskt": maskt,
        "identb": np.eye(128, dtype=np.float32).astype(BF),
    }
    if apply_g:
        shared["lng"] = np.broadcast_to(ln_g, (128, DIM)).copy()
    if apply_b:
        shared["lnb"] = np.broadcast_to(ln_b, (128, DIM)).copy()

    n_half = x.shape[1] // n_tok  # halves per batch row
    per_core = []
    for core in range(x.shape[0] * n_half):
        b = core // n_half
        h = core % n_half
        t0 = h * n_tok
        xp = np.zeros((n_tok + 128, DIM), np.float32)
        xp[128:] = x[b, t0:t0 + n_tok]
        if t0 > 0:
            xp[127] = x[b, t0 - 1]
        per_core.append({"xp": xp})
    return shared, per_core, apply_g, apply_b


def build_bass(n_tok, apply_g, apply_b, silu_native=True):
    nc = bacc.Bacc("TRN2", target_bir_lowering=False, debug=False,
                   num_devices=1)
    specs = {
        "xp": ([n_tok + 128, DIM], F32),
        "wqk": ([512, 128], BF16),
        "whvg": ([512, 1024], BF16),
        "wga": ([512, 512], BF16),
        "wcomb": ([512, 512], BF16),
        "woa": ([2048, 512], BF16),
        "g4t": ([128, 4], F32),
        "b4t": ([128, 4], F32),
        "g2t": ([128, 2], F32),
        "b2t": ([128, 2], F32),
        "maskt": ([256, 256], F32),
        "identb": ([128, 128], BF16),
    }
    if apply_g:
        specs["lng"] = ([128, 512], F32)
    if apply_b:
        specs["lnb"] = ([128, 512], F32)
    aps = {}
    for name, (shape, dt) in specs.items():
        aps[name] = nc.dram_tensor(name, shape, dt, kind="ExternalInput").ap()
    aps["y"] = nc.dram_tensor("y", [n_tok, DIM], F32,
                              kind="ExternalOutput").ap()

    with tile.TileContext(nc) as tc:
        with ExitStack() as ctx:
            build_core_program(ctx, tc, aps, n_tok, apply_g, apply_b,
                               silu_native=silu_native)
    nc.compile()
    return nc


def _run(inputs, trace=False, **spmd_kwargs):
    from concourse.bass_utils import run_bass_kernel_spmd

    shared, per_core, apply_g, apply_b = make_host_inputs(
        inputs["x"], inputs["ln_g"], inputs["ln_b"], inputs["w_qk"],
        inputs["g4"], inputs["b4"], inputs["g2"], inputs["b2"],
        inputs["w_hidden"], inputs["w_gate"], inputs["w_out"])

    nc = build_bass(T_CORE, apply_g, apply_b)

    in_maps = [{**shared, **pc} for pc in per_core]
    res = run_bass_kernel_spmd(nc, in_maps, core_ids=list(range(N_CORES)),
                               trace=trace, **spmd_kwargs)

    y = np.empty((B, SEQ, DIM), np.float32)
    n_half = SEQ // T_CORE
    for core in range(N_CORES):
        b = core // n_half
        h = core % n_half
        y[b, h * T_CORE:(h + 1) * T_CORE] = res.results[core]["y"]
    return y, res


def kernel(**inputs):
    return _run(inputs)[0]


# revision 9
# speedup vs baseline: 1.2561x; 1.2561x over previous
"""Trainium2 Bass kernel for nn_FLASH_ShareA_FFConvM.

Strategy: data-parallel over (batch, seq-half): 8 cores, each handling 4096
tokens (16 local-attention chunks of 256). All weights replicated. Per core:

  phase A (token-major): LayerNorm stats + normalized x, bf16 copy,
    DMA-transpose into channel-major xs^T [512c x 4224t] (col j = token j-128;
    128-row halo tile at the front provides the token-shift source).
    The "shift first half of channels by one token" becomes a column-offset
    AP on xs^T c-chunks 0..1.
  phase B (per chunk g): qk^T = wqk^T @ xs_sh^T; per-chunk mean stats ->
    OffsetScale -> q/k/qs/ks (ACT per-partition scale+bias, shift = col AP);
    sim^T[j,i] per head via PE (K=64); mask+exp; softmax denominator via
    ones-matmul; reciprocal broadcast via DRAM bounce; attn = exp * rec;
    V-matmul out^T[e,(h,i)] with token-major hidden as stationary;
    silu*gate -> og^T; fin[t,d] PSUM accumulates xs_sh@W_comb (folded
    hidden@w_out[:512]) + og@w_out[512:]; finalize y = fin*silu(vgate) + xs_sh
    with fp32 xs recomputed from saved LN stats (token-shift via SBUF DMA).
"""

import sys

sys.path.insert(0, "/opt/trn_rl_repo")

import numpy as np
import ml_dtypes
from contextlib import ExitStack

import concourse.bass as bass
import concourse.tile as tile
from concourse import bacc, mybir

F32 = mybir.dt.float32
BF16 = mybir.dt.bfloat16
AX = mybir.AxisListType
ALU = mybir.AluOpType
ACTF = mybir.ActivationFunctionType

B, SEQ, DIM = 4, 8192, 512
G, QK = 32, 128
CHUNK = SEQ // G          # 256 tokens per attention chunk
HD = QK // 4              # 32 (softmax scale dim, per source)
SCALE = float(HD) ** -0.5
HID = DIM                 # 512
EPS = 1e-5
N_CORES = 8
T_CORE = SEQ // 2         # 4096 tokens per core
NEG = -1.0e30

BF = ml_dtypes.bfloat16


def build_core_program(ctx: ExitStack, tc, aps, n_tok, apply_g, apply_b,
                       pe_transpose=False, silu_native=True):
    """Emit the per-core program. aps: dict name -> bass.AP (DRAM)."""
    nc = tc.nc
    n_tiles = n_tok // 128            # 128-token tiles (excl. halo tile)
    n_chunks = n_tok // CHUNK
    n_pad = n_tok + 128               # halo tile rows 0..127 (token j-128)

    xp = aps["xp"]; yout = aps["y"]

    # ---------------- pools ----------------
    consts = ctx.enter_context(tc.tile_pool(name="consts", bufs=1))
    persist = ctx.enter_context(tc.tile_pool(name="persist", bufs=1))
    work = ctx.enter_context(tc.tile_pool(name="work", bufs=1))
    psum = ctx.enter_context(tc.tile_pool(name="psum", bufs=1, space="PSUM"))
    dram = ctx.enter_context(tc.tile_pool(name="dram", bufs=1, space="DRAM"))

    # ---------------- constants into SBUF ----------------
    def cload(name, shape, dtype):
        """Load a [R, C] DRAM const into SBUF; R>128 folds to [128, R//128, C]
        (row r = cc*128 + p -> tile[p, cc, :]), returning slices via [:, cc, :].
        """
        if shape[0] > 128:
            k = shape[0] // 128
            t = consts.tile([128, k, shape[1]], dtype, name=f"c_{name}",
                            tag=f"c_{name}")
            nc.sync.dma_start(t[:], aps[name].rearrange("(k p) c -> p k c",
                                                        p=128))
        else:
            t = consts.tile(shape, dtype, name=f"c_{name}", tag=f"c_{name}")
            nc.sync.dma_start(t[:], aps[name])
        return t

    wqk = cload("wqk", [512, 128], BF16)      # [128c, 4cc, 128d]
    whvg = cload("whvg", [512, 1024], BF16)   # moving: cols 0:512 hid, 512: vgate
    wga = cload("wga", [512, 512], BF16)      # attn gate; lhsT tiles
    wcomb = cload("wcomb", [512, 512], BF16)  # moving rhs for fin part1
    woa = cload("woa", [2048, 512], BF16)     # moving rhs for fin part2
    g4t = cload("g4t", [128, 4], F32)         # pre-divided by CHUNK
    b4t = cload("b4t", [128, 4], F32)
    g2t = cload("g2t", [128, 2], F32)         # pre-divided by CHUNK
    b2t = cload("b2t", [128, 2], F32)
    maskt = cload("maskt", [256, 256], F32)   # [j, i]: 0 if j<=i else NEG
    if apply_g:
        lng = cload("lng", [128, 512], F32)
    if apply_b:
        lnb = cload("lnb", [128, 512], F32)
    identb = cload("identb", [128, 128], BF16)

    ones_bf = consts.tile([128, 1], BF16, name="ones_bf", tag="ones_bf")
    nc.vector.memset(ones_bf[:], 1.0)
    epsb = consts.tile([128, 1], F32, name="epsb", tag="epsb")
    nc.vector.memset(epsb[:], EPS)

    # ---------------- persistent state ----------------
    # channel-major normalized+g-scaled x, bf16; col j = token (j - 128)
    # folded: xsT[p, cc, j] = xs[token j-128, channel cc*128+p]
    xsT = persist.tile([128, 4, n_pad], BF16, name="xsT", tag="xsT")
    # per-LN-tile stats: cols 2i (mean), 2i+1 (rstd); tile i covers xp rows
    # [i*128, (i+1)*128)
    stats = persist.tile([128, 2 * (n_tiles + 1)], F32, name="stats",
                         tag="stats")

    # ---------------- phase A: LN + transpose ----------------
    def emit_ln_tile(i):
        x_t = work.tile([128, 512], F32, name=f"xa{i}", tag="xa", bufs=3)
        nc.sync.dma_start(x_t[:], xp[i * 128:(i + 1) * 128, :])

        bns = work.tile([128, 6], F32, name=f"bns{i}", tag="bns", bufs=2)
        nc.vector.bn_stats(out=bns[:], in_=x_t[:])
        mv = work.tile([128, 2], F32, name=f"mv{i}", tag="mv", bufs=2)
        nc.vector.bn_aggr(out=mv[:], in_=bns[:])
        nc.vector.tensor_copy(out=stats[:, 2 * i:2 * i + 1], in_=mv[:, 0:1])
        # rstd = 1/sqrt(var+eps)
        sd = work.tile([128, 1], F32, name=f"sd{i}", tag="sd", bufs=2)
        nc.scalar.activation(out=sd[:], in_=mv[:, 1:2], func=ACTF.Sqrt,
                             bias=epsb[:])
        nc.vector.reciprocal(out=sd[:], in_=sd[:])
        nc.vector.tensor_copy(out=stats[:, 2 * i + 1:2 * i + 2], in_=sd[:])

        xs_bf = work.tile([128, 512], BF16, name=f"xsbf{i}", tag="xsbf",
                          bufs=3)
        if apply_g or apply_b:
            xs_f = work.tile([128, 512], F32, name=f"xsf{i}", tag="xsf",
                             bufs=2)
            nc.vector.tensor_scalar(out=xs_f[:], in0=x_t[:],
                                    scalar1=mv[:, 0:1], scalar2=sd[:],
                                    op0=ALU.subtract, op1=ALU.mult)
            if apply_g:
                nc.vector.tensor_mul(out=(xs_f if apply_b else xs_bf)[:],
                                     in0=xs_f[:], in1=lng[:])
            if apply_b:
                nc.vector.tensor_add(out=xs_bf[:], in0=xs_f[:], in1=lnb[:])
        else:
            nc.vector.tensor_scalar(out=xs_bf[:], in0=x_t[:],
                                    scalar1=mv[:, 0:1], scalar2=sd[:],
                                    op0=ALU.subtract, op1=ALU.mult)

        # transpose [128t, 512c] -> xsT[:, :, i*128:(i+1)*128] in one DMA
        if pe_transpose:
            for cc in range(4):
                tp = psum.tile([128, 128], BF16, name=f"tp{i}_{cc}",
                               tag="mm_s", bufs=2)
                nc.tensor.transpose(tp[:], xs_bf[:, cc * 128:(cc + 1) * 128],
                                    identb[:])
                nc.vector.tensor_copy(out=xsT[:, cc, i * 128:(i + 1) * 128],
                                      in_=tp[:])
        else:
            nc.sync.dma_start(xsT[:, :, i * 128:(i + 1) * 128], xs_bf[:],
                              transpose=True)

    # ---------------- halo xs (token-major, fp32) for residual shift -------
    def emit_xs_tok(ti):
        """Recompute fp32 token-major xs for xp tile ti (rows ti*128..)."""
        x2 = work.tile([128, 512], F32, name=f"xr{ti}", tag="xr", bufs=3)
        nc.sync.dma_start(x2[:], xp[ti * 128:(ti + 1) * 128, :])
        xs = work.tile([128, 512], F32, name=f"xstok{ti}", tag="xstok",
                       bufs=3)
        nc.vector.tensor_scalar(out=xs[:], in0=x2[:],
                                scalar1=stats[:, 2 * ti:2 * ti + 1],
                                scalar2=stats[:, 2 * ti + 1:2 * ti + 2],
                                op0=ALU.subtract, op1=ALU.mult)
        if apply_g:
            nc.vector.tensor_mul(out=xs[:], in0=xs[:], in1=lng[:])
        if apply_b:
            nc.vector.tensor_add(out=xs[:], in0=xs[:], in1=lnb[:])
        return xs

    # interleave: LN tiles stay a couple ahead of the chunk that needs them
    ln_next = 0

    def ensure_ln(upto):
        nonlocal ln_next
        while ln_next <= min(upto, n_tiles):
            emit_ln_tile(ln_next)
            ln_next += 1

    ensure_ln(2)
    prev_xs = emit_xs_tok(0)   # halo tile; only row 127 is ever read

    # ---------------- phase B: chunks ----------------
    for g in range(n_chunks):
        ensure_ln(2 * g + 4)
        colU = 128 + g * CHUNK        # unshifted start col in xsT
        colS = colU - 1               # shifted (first-half channels)

        def xsh(cc, col0, width):
            c0 = col0 - 1 if cc < 2 else col0
            return xsT[:, cc, c0:c0 + width]

        # --- B1: qk^T [128qk, 256] ---
        qkps = psum.tile([128, 256], F32, name=f"qkps{g}", tag="mm_s", bufs=2)
        for cc in range(4):
            nc.tensor.matmul(qkps[:], wqk[:, cc, :],
                             xsh(cc, colU, 256),
                             start=(cc == 0), stop=(cc == 3))
        qkT = work.tile([128, 256], F32, name=f"qkT{g}", tag="qkT", bufs=2)
        nc.vector.tensor_copy(out=qkT[:], in_=qkps[:])

        # --- B2: per-chunk stats -> offsets/scales [128,1] each ---
        qsum = work.tile([128, 1], F32, name=f"qsum{g}", tag="qsum", bufs=2)
        nc.vector.tensor_reduce(out=qsum[:], in_=qkT[:], axis=AX.X,
                                op=ALU.add)
        offs = work.tile([128, 6], F32, name=f"offs{g}", tag="offs", bufs=2)
        for idx in range(4):   # 0 qoff, 1 koff, 2 qsc, 3 ksc
            nc.vector.tensor_scalar(out=offs[:, idx:idx + 1], in0=qsum[:],
                                    scalar1=g4t[:, idx:idx + 1],
                                    scalar2=b4t[:, idx:idx + 1],
                                    op0=ALU.mult, op1=ALU.add)
        for idx in range(2):   # 4 qsoff, 5 ksoff
            nc.vector.tensor_scalar(out=offs[:, 4 + idx:5 + idx],
                                    in0=qsum[:],
                                    scalar1=g2t[:, idx:idx + 1],
                                    scalar2=b2t[:, idx:idx + 1],
                                    op0=ALU.mult, op1=ALU.add)
        qoff, koff, qsc, ksc, qsoff, ksoff = (offs[:, i:i + 1]
                                              for i in range(6))

        # --- B3: q/k/qs/ks channel-major bf16 [128qk, 256] ---
        qT = work.tile([128, 256], BF16, name=f"qT{g}", tag="qT", bufs=2)
        kT = work.tile([128, 256], BF16, name=f"kT{g}", tag="kT", bufs=2)
        nc.vector.tensor_scalar(out=qT[:], in0=qkT[:], scalar1=qsc,
                                scalar2=qoff, op0=ALU.mult, op1=ALU.add)
        nc.vector.tensor_scalar(out=kT[:], in0=qkT[:], scalar1=ksc,
                                scalar2=koff, op0=ALU.mult, op1=ALU.add)
        qsT = work.tile([128, 256], BF16, name=f"qsT{g}", tag="qsT", bufs=2)
        ksT = work.tile([128, 256], BF16, name=f"ksT{g}", tag="ksT", bufs=2)
        nc.vector.tensor_copy(out=qsT[:, 0:1], in_=qsoff)
        nc.vector.tensor_copy(out=ksT[:, 0:1], in_=ksoff)
        nc.vector.tensor_scalar(out=qsT[:, 1:256], in0=qT[:, 0:255],
                                scalar1=qsc, scalar2=qsoff, op0=ALU.mult,
                                op1=ALU.add)
        nc.vector.tensor_scalar(out=ksT[:, 1:256], in0=kT[:, 0:255],
                                scalar1=ksc, scalar2=ksoff, op0=ALU.mult,
                                op1=ALU.add)

        # --- B4: sim^T + mask + exp -> exp[jt] [128j, 4h*256i] bf16 ---
        expt = [work.tile([128, 1024], BF16, name=f"exp{g}_{jt}",
                          tag=f"exp{jt}", bufs=2) for jt in range(2)]
        for h in range(4):
            Q = qT if h < 2 else qsT
            K = kT if h < 2 else ksT
            dr = (h % 2) * 64
            for jt in range(2):
                sim = psum.tile([128, 256], F32, name=f"sim{g}_{h}_{jt}",
                                tag="mm_s", bufs=2)
                nc.tensor.matmul(sim[:],
                                 K[dr:dr + 64, jt * 128:(jt + 1) * 128],
                                 Q[dr:dr + 64, :], start=True, stop=True)
                tmp = work.tile([128, 256], F32, name=f"ms{g}_{h}_{jt}",
                                tag="ms", bufs=3)
                nc.vector.scalar_tensor_tensor(
                    out=tmp[:], in0=sim[:], scalar=SCALE,
                    in1=maskt[:, jt, :],
                    op0=ALU.mult, op1=ALU.add)
                nc.scalar.activation(out=expt[jt][:, h * 256:(h + 1) * 256],
                                     in_=tmp[:], func=ACTF.Exp)

        # --- B5: denominator + reciprocal + broadcast + attn ---
        sums = psum.tile([1, 1024], F32, name=f"sums{g}", tag="hvv", bufs=2)
        for jt in range(2):
            for s in range(2):
                nc.tensor.matmul(sums[0:1, s * 512:(s + 1) * 512], ones_bf[:],
                                 expt[jt][:, s * 512:(s + 1) * 512],
                                 start=(jt == 0), stop=(jt == 1))
        rec = work.tile([1, 1024], F32, name=f"rec{g}", tag="rec", bufs=2)
        nc.vector.reciprocal_approx_fast(out=rec[:], in_=sums[0:1, :])
        recd = dram.tile([1, 1024], F32, name=f"recd{g}", tag="recd", bufs=2)
        nc.sync.dma_start(recd[:], rec[:])
        recb = work.tile([128, 1024], F32, name=f"recb{g}", tag="recb",
                         bufs=2)
        bcast = bass.AP(tensor=recd.tensor, offset=recd.offset,
                        ap=[[0, 128], list(recd.ap)[-1]])
        nc.sync.dma_start(recb[:], bcast)
        attn = [work.tile([128, 1024], BF16, name=f"attn{g}_{jt}",
                          tag=f"attn{jt}", bufs=2) for jt in range(2)]
        for jt in range(2):
            nc.vector.tensor_mul(out=attn[jt][:], in0=expt[jt][:],
                                 in1=recb[:])

        # --- B6: hidden + vgate (token-major) for the chunk's 2 t-tiles ---
        hid_bf = []
        svg_bf = []
        for tt in range(2):
            ti = g * 2 + tt
            colT = 128 + ti * 128
            hv = psum.tile([128, 1024], F32, name=f"hv{g}_{tt}", tag="hvv",
                           bufs=2)
            for cc in range(4):
                for s in range(2):
                    nc.tensor.matmul(
                        hv[:, s * 512:(s + 1) * 512],
                        xsh(cc, colT, 128),
                        whvg[:, cc, s * 512:(s + 1) * 512],
                        start=(cc == 0), stop=(cc == 3))
            hb = work.tile([128, 512], BF16, name=f"hid{g}_{tt}", tag="hid",
                           bufs=4)
            nc.vector.tensor_copy(out=hb[:], in_=hv[:, 0:512])
            sv = work.tile([128, 512], BF16, name=f"svg{g}_{tt}", tag="svg",
                           bufs=4)
            if silu_native:
                nc.scalar.activation(out=sv[:], in_=hv[:, 512:1024],
                                     func=ACTF.Silu)
            else:
                sgt = work.tile([128, 512], F32, name=f"sg{g}_{tt}",
                                tag="sgt", bufs=2)
                nc.scalar.activation(out=sgt[:], in_=hv[:, 512:1024],
                                     func=ACTF.Sigmoid)
                nc.vector.tensor_mul(out=sv[:], in0=sgt[:],
                                     in1=hv[:, 512:1024])
            hid_bf.append(hb)
            svg_bf.append(sv)

        # --- B7: attn gate^T (channel-major) ---
        gate_bf = []
        for ee in range(4):
            gps = psum.tile([128, 256], F32, name=f"g{g}_{ee}", tag="mm_s",
                            bufs=2)
            for cc in range(4):
                nc.tensor.matmul(gps[:],
                                 wga[:, cc, ee * 128:(ee + 1) * 128],
                                 xsh(cc, colU, 256),
                                 start=(cc == 0), stop=(cc == 3))
            gb = work.tile([128, 256], BF16, name=f"gate{g}_{ee}",
                           tag=f"gate{ee}", bufs=2)
            nc.vector.tensor_copy(out=gb[:], in_=gps[:])
            gate_bf.append(gb)

        # --- B8: V matmul + silu + gate -> og^T [128e, 4h*256i] bf16 ---
        og_bf = []
        for ee in range(4):
            vps = psum.tile([128, 1024], F32, name=f"v{g}_{ee}", tag="hvv",
                            bufs=2)
            for jt in range(2):
                for s in range(2):
                    nc.tensor.matmul(vps[:, s * 512:(s + 1) * 512],
                                     hid_bf[jt][:, ee * 128:(ee + 1) * 128],
                                     attn[jt][:, s * 512:(s + 1) * 512],
                                     start=(jt == 0), stop=(jt == 1))
            osl = work.tile([128, 1024], BF16, name=f"osl{g}_{ee}",
                            tag="osl", bufs=2)
            if silu_native:
                nc.scalar.activation(out=osl[:], in_=vps[:], func=ACTF.Silu)
            else:
                sgo = work.tile([128, 1024], F32, name=f"sgo{g}_{ee}",
                                tag="sgo", bufs=2)
                nc.scalar.activation(out=sgo[:], in_=vps[:],
                                     func=ACTF.Sigmoid)
                nc.vector.tensor_mul(out=osl[:], in0=sgo[:], in1=vps[:])
            ob = work.tile([128, 1024], BF16, name=f"og{g}_{ee}",
                           tag=f"og{ee}", bufs=2)
            gbc = gate_bf[ee].unsqueeze(1).broadcast_to((128, 4, 256))
            nc.vector.tensor_mul(out=ob.rearrange("p (h i) -> p h i", h=4),
                                 in0=osl.rearrange("p (h i) -> p h i", h=4),
                                 in1=gbc)
            og_bf.append(ob)

        # --- B9 + B10: fin PSUM accumulation, then finalize each t-tile ---
        for tt in range(2):
            ti = g * 2 + tt
            colT = 128 + ti * 128
            fin = psum.tile([128, 512], F32, name=f"fin{g}_{tt}", tag="fin",
                            bufs=2)
            for cc in range(4):
                nc.tensor.matmul(fin[:], xsh(cc, colT, 128),
                                 wcomb[:, cc, :],
                                 start=(cc == 0), stop=False)
            for h in range(4):
                for ee in range(4):
                    ff = h * 4 + ee
                    nc.tensor.matmul(
                        fin[:],
                        og_bf[ee][:, h * 256 + tt * 128:h * 256 + tt * 128
                                  + 128],
                        woa[:, ff, :],
                        start=False, stop=(ff == 15))

            xs_cur = emit_xs_tok(ti + 1)
            xsprev = work.tile([128, 256], F32, name=f"xsp{ti}", tag="xsp",
                               bufs=2)
            nc.sync.dma_start(xsprev[1:128, :], xs_cur[0:127, 0:256])
            nc.sync.dma_start(xsprev[0:1, :], prev_xs[127:128, 0:256])
            prev_xs = xs_cur

            y = work.tile([128, 512], F32, name=f"y{ti}", tag="y", bufs=3)
            nc.vector.tensor_mul(out=y[:], in0=fin[:], in1=svg_bf[tt][:])
            nc.vector.tensor_add(out=y[:, 256:512], in0=y[:, 256:512],
                                 in1=xs_cur[:, 256:512])
            nc.vector.tensor_add(out=y[:, 0:256], in0=y[:, 0:256],
                                 in1=xsprev[:])
            nc.sync.dma_start(yout[ti * 128:(ti + 1) * 128, :], y[:])


def make_host_inputs(x, ln_g, ln_b, w_qk, g4, b4, g2, b2, w_hidden, w_gate,
                     w_out, n_tok=T_CORE):
    """Build shared weight arrays + per-core xp slices."""
    x = np.asarray(x, np.float32)
    ln_g = np.asarray(ln_g, np.float32)
    ln_b = np.asarray(ln_b, np.float32)
    apply_g = not np.all(ln_g == 1.0)
    apply_b = bool(np.any(ln_b != 0.0))

    w_hidden = np.asarray(w_hidden, np.float32)
    w_out = np.asarray(w_out, np.float32)
    w_gate = np.asarray(w_gate, np.float32)
    w_qk = np.asarray(w_qk, np.float32)

    wcomb = (w_hidden[:, :HID] @ w_out[:HID, :]).astype(np.float32)

    jj, ii = np.meshgrid(np.arange(256), np.arange(256), indexing="ij")
    maskt = np.where(jj > ii, np.float32(NEG), np.float32(0.0))

    shared = {
        "wqk": w_qk.astype(BF),
        "whvg": np.concatenate([w_hidden[:, :HID], w_gate], axis=1).astype(BF),
        "wga": w_hidden[:, HID:].astype(BF),
        "wcomb": wcomb.astype(BF),
        "woa": w_out[HID:, :].astype(BF),
        "g4t": np.broadcast_to((np.asarray(g4, np.float32) / CHUNK).T,
                               (QK, 4)).copy(),
        "b4t": np.broadcast_to(np.asarray(b4, np.float32).T, (QK, 4)).copy(),
        "g2t": np.broadcast_to((np.asarray(g2, np.float32) / CHUNK).T,
                               (QK, 2)).copy(),
        "b2t": np.broadcast_to(np.asarray(b2, np.float32).T, (QK, 2)).copy(),
        "maskt": maskt,
        "identb": np.eye(128, dtype=np.float32).astype(BF),
    }
    if apply_g:
        shared["lng"] = np.broadcast_to(ln_g, (128, DIM)).copy()
    if apply_b:
        shared["lnb"] = np.broadcast_to(ln_b, (128, DIM)).copy()

    n_half = x.shape[1] // n_tok  # halves per batch row
    per_core = []
    for core in range(x.shape[0] * n_half):
        b = core // n_half
        h = core % n_half
        t0 = h * n_tok
        xp = np.zeros((n_tok + 128, DIM), np.float32)
        xp[128:] = x[b, t0:t0 + n_tok]
        if t0 > 0:
            xp[127] = x[b, t0 - 1]
        per_core.append({"xp": xp})
    return shared, per_core, apply_g, apply_b


def build_bass(n_tok, apply_g, apply_b, silu_native=True):
    nc = bacc.Bacc("TRN2", target_bir_lowering=False, debug=False,
                   num_devices=1)
    specs = {
        "xp": ([n_tok + 128, DIM], F32),
        "wqk": ([512, 128], BF16),
        "whvg": ([512, 1024], BF16),
        "wga": ([512, 512], BF16),
        "wcomb": ([512, 512], BF16),
        "woa": ([2048, 512], BF16),
        "g4t": ([128, 4], F32),
        "b4t": ([128, 4], F32),
        "g2t": ([128, 2], F32),
        "b2t": ([128, 2], F32),
        "maskt": ([256, 256], F32),
        "identb": ([128, 128], BF16),
    }
    if apply_g:
        specs["lng"] = ([128, 512], F32)
    if apply_b:
        specs["lnb"] = ([128, 512], F32)
    aps = {}
    for name, (shape, dt) in specs.items():
        aps[name] = nc.dram_tensor(name, shape, dt, kind="ExternalInput").ap()
    aps["y"] = nc.dram_tensor("y", [n_tok, DIM], F32,
                              kind="ExternalOutput").ap()

    with tile.TileContext(nc) as tc:
        with ExitStack() as ctx:
            build_core_program(ctx, tc, aps, n_tok, apply_g, apply_b,
                               silu_native=silu_native)
    nc.compile()
    return nc


def _run(inputs, trace=False, **spmd_kwargs):
    from concourse.bass_utils import run_bass_kernel_spmd

    shared, per_core, apply_g, apply_b = make_host_inputs(
        inputs["x"], inputs["ln_g"], inputs["ln_b"], inputs["w_qk"],
        inputs["g4"], inputs["b4"], inputs["g2"], inputs["b2"],
        inputs["w_hidden"], inputs["w_gate"], inputs["w_out"])

    nc = build_bass(T_CORE, apply_g, apply_b)

    in_maps = [{**shared, **pc} for pc in per_core]
    res = run_bass_kernel_spmd(nc, in_maps, core_ids=list(range(N_CORES)),
                               trace=trace, **spmd_kwargs)

    y = np.empty((B, SEQ, DIM), np.float32)
    n_half = SEQ // T_CORE
    for core in range(N_CORES):
        b = core // n_half
        h = core % n_half
        y[b, h * T_CORE:(h + 1) * T_CORE] = res.results[core]["y"]
    return y, res


def kernel(**inputs):
    return _run(inputs)[0]
